# revision 1
# baseline (speedup 1.0000x reference)
"""Trainium2 Bass kernel for nn_EncoderTransformer_61194694033513.

Data-parallel over batch B=16 across 8 NeuronCores (2 batch elems per core).
Per core, the whole forward runs out of SBUF with activations stored
feature-major HT[e, tok] in fp16 (matmul operands must be 16-bit to stream at
1 column/cycle on the PE; fp32 matmul runs at 1/4 rate; fp16 carries 10
mantissa bits vs bf16's 7, and squares are pre-scaled by 1/64 to stay in
fp16 range). All matmul
accumulation is fp32 in PSUM. Attention is computed flash-style (S^T tiles of
[128 keys x 512 queries], relu, accumulated into O^T) so the [N,N] matrix is
never materialized. LayerNorm reductions over the feature (partition) axis go
through the PE with a ones lhsT into [1,512] PSUM rows; the per-token row math
runs in fp32 on partition 0, rstd is computed as exp(-0.5*ln(var+eps)) on the
scalar engine (one table set, no slow DVE reciprocal), and rstd / mean*rstd
rows are broadcast back over partitions with K=1 matmuls whose lhsT carries
g / -g. The apply is one tensor_tensor + one scalar_tensor_tensor per 128x512
block (beta rides in as the per-partition fp32 scalar).
"""

import sys

import numpy as np

for _p in (
    "/opt/trn_rl_repo",
    "/opt/pypackages",
    "/root/.axon_site",
    "/root/.axon_site/_ro/trn_rl_repo",
    "/root/.axon_site/_ro/pypackages",
):
    if _p not in sys.path:
        sys.path.append(_p)

import ml_dtypes  # noqa: E402

import concourse.bass as bass  # noqa: E402
import concourse.bacc as bacc  # noqa: E402
import concourse.mybir as mybir  # noqa: E402
from concourse import tile  # noqa: E402
from concourse.bass_utils import run_bass_kernel_spmd  # noqa: E402

B, N, D, E, L = 16, 2048, 128, 256, 3
NCORES = 8
BL = B // NCORES  # batch elems per core
P = 128
EC = E // P  # feature-dim partition chunks (2)
CH = N // 512  # 512-wide token chunks (4)
JT = N // P  # key tiles (16)
EPS = 1e-5
F32 = mybir.dt.float32
F16 = mybir.dt.float16
NPF16 = np.float16
AF = mybir.ActivationFunctionType
OP = mybir.AluOpType

_CACHE = {}


def _build():
    nc = bacc.Bacc("TRN2", target_bir_lowering=False, debug=False, num_devices=NCORES)

    d_xsT = nc.declare_dram_parameter("xsT", [BL, P, N], F16, isOutput=False)
    d_Win = nc.declare_dram_parameter("Win", [D, E], F16, isOutput=False)
    d_W = {
        nm: nc.declare_dram_parameter(nm, [L, E, E], F16, isOutput=False)
        for nm in ("Wq", "Wk", "Wv", "W1", "W2")
    }
    d_WoutT = nc.declare_dram_parameter("WoutT", [E, N], F32, isOutput=False)
    d_colpack = nc.declare_dram_parameter("colpack", [P, 2 + 6 * L * EC], F32, isOutput=False)
    d_bout = nc.declare_dram_parameter("b_out", [1, 1], F32, isOutput=False)
    d_out = nc.declare_dram_parameter("out", [BL, 1], F32, isOutput=True)

    with tile.TileContext(nc) as tc:
        from contextlib import ExitStack

        with ExitStack() as ctx:
            cpool = ctx.enter_context(tc.tile_pool(name="const", bufs=1))
            hpool = ctx.enter_context(tc.tile_pool(name="acts", bufs=1))
            xs_pool = ctx.enter_context(tc.tile_pool(name="xs", bufs=2))
            spool = ctx.enter_context(tc.tile_pool(name="srelu", bufs=3))
            sqpool = ctx.enter_context(tc.tile_pool(name="sqp", bufs=4))
            apool = ctx.enter_context(tc.tile_pool(name="mlpa", bufs=3))
            tpool = ctx.enter_context(tc.tile_pool(name="t1p", bufs=2))
            bcpool = ctx.enter_context(tc.tile_pool(name="bc", bufs=4))
            ropool = ctx.enter_context(tc.tile_pool(name="ro", bufs=2))

            PS = bass.MemorySpace.PSUM
            ps_s = ctx.enter_context(tc.tile_pool(name="ps_s", bufs=2, space=PS))
            ps_o = ctx.enter_context(tc.tile_pool(name="ps_o", bufs=2, space=PS))
            ps_mm = ctx.enter_context(tc.tile_pool(name="ps_mm", bufs=2, space=PS))

            # ---- input DMAs first so the input projection can start
            # while weights stream in ------------------------------------
            xs_tiles = []
            for b in range(BL):
                xt = xs_pool.tile([P, N], F16, name=f"xst{b}", tag=f"xst{b}")
                nc.sync.dma_start(xt[:], d_xsT[b])
                xs_tiles.append(xt)

            # ---- constants / weights -------------------------------------
            win_sb = cpool.tile([P, E], F16, name="win", tag="win")
            nc.sync.dma_start(win_sb[:], d_Win[:])
            # one DMA per (name, ec) loads all L layers into a [P, L*E] tile
            w_big = {}
            w_sb = {nm: [None] * L for nm in ("Wq", "Wk", "Wv", "W1", "W2")}
            for nm in ("Wq", "Wk", "Wv", "W1", "W2"):
                w_big[nm] = []
                for ec in range(EC):
                    t = cpool.tile([P, L * E], F16, name=f"{nm}B{ec}", tag=f"{nm}B{ec}")
                    nc.sync.dma_start(
                        t[:].rearrange("p (l e) -> p l e", l=L),
                        d_W[nm][:, ec * P : (ec + 1) * P, :].rearrange("l p e -> p l e"),
                    )
                    w_big[nm].append(t)
            for nm in w_big:
                for l in range(L):
                    w_sb[nm][l] = [
                        w_big[nm][ec][:, l * E : (l + 1) * E] for ec in range(EC)
                    ]

            colpack = cpool.tile([P, 2 + 6 * L * EC], F32, name="colpack", tag="colpack")
            nc.sync.dma_start(colpack[:], d_colpack[:])
            binp_sb = colpack[:, 0:EC]

            def col_views(base):
                return [
                    colpack[:, 2 + base * L * EC + l * EC : 2 + base * L * EC + (l + 1) * EC]
                    for l in range(L)
                ]

            bm1_sb = col_views(0)
            bm2_sb = col_views(1)
            be1_sb = col_views(2)
            be2_sb = col_views(3)
            g1_sb = col_views(4)
            g2_sb = col_views(5)
            bout_sb = cpool.tile([1, 1], F32, name="bout", tag="bout")
            nc.sync.dma_start(bout_sb[:], d_bout[:])
            wout_sb = []
            for ec in range(EC):
                t = cpool.tile([P, N], F32, name=f"wout{ec}", tag=f"wout{ec}")
                nc.sync.dma_start(t[:], d_WoutT[ec * P : (ec + 1) * P, :])
                wout_sb.append(t)

            ones_kb = cpool.tile([P, 1], F16, name="ones_kb", tag="ones_kb")
            nc.vector.memset(ones_kb[:], 1.0)
            ones_kf = cpool.tile([P, 1], F32, name="ones_kf", tag="ones_kf")
            nc.vector.memset(ones_kf[:], 1.0)
            eps1 = cpool.tile([1, 1], F32, name="eps1", tag="eps1")
            nc.vector.memset(eps1[:], EPS)

            # LN row scratch: partition 0. rowsF fp32 (sum / sumsq / var),
            # rowsB bf16 (rstd / mean*rstd) for the broadcast matmul rhs.
            rowsF = cpool.tile([1, 2 * N], F32, name="rowsF", tag="rowsF")
            rowsB = cpool.tile([1, 2 * N], F16, name="rowsB", tag="rowsB")
            rstd_row = rowsB[:, 0:N]
            mrstd_row = rowsB[:, N : 2 * N]

            # ---- persistent activations (fp16), one set per batch elem ----
            Hf = [[hpool.tile([P, N], F16, name=f"Hf{b}{ec}", tag=f"Hf{b}{ec}") for ec in range(EC)] for b in range(BL)]
            qT = [[hpool.tile([P, N], F16, name=f"qT{b}{dc}", tag=f"qT{b}{dc}") for dc in range(EC)] for b in range(BL)]
            kT = [[hpool.tile([P, N], F16, name=f"kT{b}{dc}", tag=f"kT{b}{dc}") for dc in range(EC)] for b in range(BL)]
            v_sb = [hpool.tile([P, JT * E], F16, name=f"v{b}", tag=f"v{b}") for b in range(BL)]

            def layernorm(X, g_col, be_col):
                """In-place LN over the feature axis of X (list of 2 [P,N] fp16
                tiles). g_col/be_col: [P,EC] fp32 per-partition params.
                Stats + row math are chunked 512 wide; rstd / mean*rstd rows are
                partition-broadcast by DMA (step-0 AP) into fp16 SBUF tiles so
                the apply runs in the DVE 16-bit fast modes."""
                for c in range(CH):
                    cs = slice(c * 512, (c + 1) * 512)
                    sqc = []
                    for pt in range(EC):
                        sq = sqpool.tile([P, 512], F16, name="sq", tag="sq")
                        nc.scalar.activation(sq[:], X[pt][:, cs], AF.Square, scale=1.0 / 64)
                        sqc.append(sq)
                    st_s = ps_mm.tile([1, 512], F32, name="st_s", tag="mm")
                    nc.tensor.matmul(st_s[:], ones_kb[:], X[0][:, cs], start=True, stop=False)
                    nc.tensor.matmul(st_s[:], ones_kb[:], X[1][:, cs], start=False, stop=True)
                    st_q = ps_mm.tile([1, 512], F32, name="st_q", tag="mm")
                    nc.tensor.matmul(st_q[:], ones_kb[:], sqc[0][:], start=True, stop=False)
                    nc.tensor.matmul(st_q[:], ones_kb[:], sqc[1][:], start=False, stop=True)
                    # chunk row math on partition 0 (fp32): sum -> SBUF, then
                    # var/4096 = stq - s0^2/(E*4096), rstd, mean*rstd
                    s0c = rowsF[:, c * 512 : (c + 1) * 512]
                    xrc = rowsF[:, N + c * 512 : N + (c + 1) * 512]
                    nc.scalar.copy(s0c, st_s[:])
                    nc.vector.tensor_mul(xrc, s0c, s0c)
                    nc.vector.scalar_tensor_tensor(
                        xrc, xrc, -1.0 / (E * 4096.0), st_q[:], op0=OP.mult, op1=OP.add
                    )
                    nc.scalar.activation(
                        rstd_row[:, cs], xrc, AF.Abs_reciprocal_sqrt,
                        bias=eps1[:], scale=4096.0 / E,
                    )
                    nc.vector.scalar_tensor_tensor(
                        mrstd_row[:, cs], s0c, 1.0 / E, rstd_row[:, cs],
                        op0=OP.mult, op1=OP.mult,
                    )
                for c2 in range(CH // 2):
                    cs2 = slice(c2 * 1024, (c2 + 1) * 1024)
                    rb = bcpool.tile([P, 1024], F16, name="rb", tag="rb")
                    nc.gpsimd.partition_broadcast(rb[:], rstd_row[:, cs2])
                    mb = bcpool.tile([P, 1024], F16, name="mb", tag="mb")
                    nc.gpsimd.partition_broadcast(mb[:], mrstd_row[:, cs2])
                    for pt in range(EC):
                        t1 = tpool.tile([P, 1024], F16, name="t1", tag="t1")
                        nc.vector.tensor_mul(t1[:], X[pt][:, cs2], rb[:])
                        t2 = tpool.tile([P, 1024], F16, name="t2", tag="t2")
                        nc.vector.tensor_sub(t2[:], t1[:], mb[:])
                        # X = (x*rstd - mean*rstd)*g + be
                        nc.vector.tensor_scalar(
                            X[pt][:, cs2], t2[:],
                            g_col[:, pt : pt + 1], be_col[:, pt : pt + 1],
                            op0=OP.mult, op1=OP.add,
                        )

            # ---- per batch element, layers interleaved across the two batch
            # elems so one stream's LN row math overlaps the other's matmuls --
            def input_proj(b):
                xs_t = xs_tiles[b]
                for ec in range(EC):
                    es = slice(ec * P, (ec + 1) * P)
                    for c in range(CH):
                        cs = slice(c * 512, (c + 1) * 512)
                        ps = ps_mm.tile([P, 512], F32, name="psin", tag="mm")
                        nc.tensor.matmul(ps[:], win_sb[:, es], xs_t[:, cs])
                        nc.vector.tensor_scalar_add(Hf[b][ec][:, cs], ps[:], binp_sb[:, ec : ec + 1])

            def qkv(b, l):
                for w_name, dstT in (("Wk", kT[b]), ("Wq", qT[b])):
                    for dc in range(EC):
                        ds_ = slice(dc * P, (dc + 1) * P)
                        for c in range(CH):
                            cs = slice(c * 512, (c + 1) * 512)
                            ps = ps_mm.tile([P, 512], F32, name="psqk", tag="mm")
                            for ec in range(EC):
                                nc.tensor.matmul(
                                    ps[:],
                                    w_sb[w_name][l][ec][:, ds_],
                                    Hf[b][ec][:, cs],
                                    start=(ec == 0),
                                    stop=(ec == EC - 1),
                                )
                            nc.scalar.copy(dstT[dc][:, cs], ps[:])
                for t in range(JT):
                    ps = ps_mm.tile([P, E], F32, name="psv", tag="mm")
                    for ec in range(EC):
                        nc.tensor.matmul(
                            ps[:],
                            Hf[b][ec][:, t * P : (t + 1) * P],
                            w_sb["Wv"][l][ec][:],
                            start=(ec == 0),
                            stop=(ec == EC - 1),
                        )
                    if t % 2 == 0:
                        nc.scalar.copy(v_sb[b][:, t * E : (t + 1) * E], ps[:])
                    else:
                        nc.vector.tensor_copy(v_sb[b][:, t * E : (t + 1) * E], ps[:])

            def attention(b):
                for c in range(CH):
                    cs = slice(c * 512, (c + 1) * 512)
                    o_ps = [
                        ps_o.tile([P, 512], F32, name=f"o{oc}", tag="o")
                        for oc in range(EC)
                    ]
                    for j2 in range(JT // 2):
                        s_ps = ps_s.tile([P, 1024], F32, name="s_ps", tag="s")
                        # one Nf=1024 matmul per (key-pair, d-chunk): rhs is the
                        # same qT 512-chunk for both key tiles via a 3D AP
                        for h in range(2):
                            j = 2 * j2 + h
                            hs = slice(h * 512, (h + 1) * 512)
                            for dc in range(EC):
                                nc.tensor.matmul(
                                    s_ps[:, hs],
                                    kT[b][dc][:, j * P : (j + 1) * P],
                                    qT[b][dc][:, cs],
                                    start=(dc == 0),
                                    stop=(dc == EC - 1),
                                )
                        sr = spool.tile([P, 1024], F16, name="sr", tag="sr")
                        if j2 % 3 == 2:
                            nc.vector.tensor_relu(sr[:], s_ps[:])
                        else:
                            nc.scalar.activation(sr[:], s_ps[:], AF.Relu)
                        for h in range(2):
                            j = 2 * j2 + h
                            hs = slice(h * 512, (h + 1) * 512)
                            for oc in range(EC):
                                nc.tensor.matmul(
                                    o_ps[oc][:],
                                    v_sb[b][:, j * E + oc * P : j * E + (oc + 1) * P],
                                    sr[:, hs],
                                    start=(j == 0),
                                    stop=(j == JT - 1),
                                )
                    for oc in range(EC):
                        nc.vector.tensor_add(Hf[b][oc][:, cs], Hf[b][oc][:, cs], o_ps[oc][:])

            def mlp(b, l):
                for c in range(CH):
                    cs = slice(c * 512, (c + 1) * 512)
                    a_t = []
                    for mc in range(EC):
                        ms = slice(mc * P, (mc + 1) * P)
                        ps = ps_mm.tile([P, 512], F32, name="psa", tag="mm")
                        for ec in range(EC):
                            nc.tensor.matmul(
                                ps[:],
                                w_sb["W1"][l][ec][:, ms],
                                Hf[b][ec][:, cs],
                                start=(ec == 0),
                                stop=(ec == EC - 1),
                            )
                        a = apool.tile([P, 512], F16, name="a", tag="a")
                        nc.scalar.activation(
                            a[:], ps[:], AF.Relu, bias=bm1_sb[l][:, mc : mc + 1]
                        )
                        a_t.append(a)
                    for oc in range(EC):
                        os_ = slice(oc * P, (oc + 1) * P)
                        ps = ps_mm.tile([P, 512], F32, name="psm", tag="mm")
                        for mc in range(EC):
                            nc.tensor.matmul(
                                ps[:],
                                w_sb["W2"][l][mc][:, os_],
                                a_t[mc][:],
                                start=(mc == 0),
                                stop=(mc == EC - 1),
                            )
                        nc.vector.scalar_tensor_tensor(
                            Hf[b][oc][:, cs],
                            ps[:],
                            bm2_sb[l][:, oc : oc + 1],
                            Hf[b][oc][:, cs],
                            op0=OP.add,
                            op1=OP.add,
                        )

            def readout(b):
                # token-partial row: psum[0, t] accumulates sum_e H[e,t]*W[e,t]
                # over both partition tiles via fp32 ones-reduce matmuls
                prods = []
                for ec in range(EC):
                    ros = ropool.tile([P, N], F32, name="ros", tag="ros")
                    nc.vector.tensor_mul(ros[:], Hf[b][ec][:], wout_sb[ec][:])
                    prods.append(ros)
                rrow = ropool.tile([1, N], F32, name="rrow", tag="rrow")
                for c in range(CH):
                    cs = slice(c * 512, (c + 1) * 512)
                    st = ps_mm.tile([1, 512], F32, name="psro", tag="mm")
                    for ec in range(EC):
                        nc.tensor.matmul(
                            st[:], ones_kf[:], prods[ec][:, cs],
                            start=(ec == 0), stop=(ec == EC - 1),
                        )
                    nc.scalar.copy(rrow[:, cs], st[:])
                rsc = ropool.tile([1, 1], F32, name="rsc", tag="rsc")
                nc.vector.reduce_sum(rsc[:], rrow[:], axis=mybir.AxisListType.X)
                ob = ropool.tile([1, 1], F32, name="ob", tag="ob")
                nc.scalar.activation(ob[:], rsc[:], AF.Identity, bias=bout_sb[:])
                nc.sync.dma_start(d_out[b : b + 1, :], ob[:])

            for b in range(BL):
                input_proj(b)
            for l in range(L):
                for b in range(BL):
                    qkv(b, l)
                    attention(b)
                    layernorm(Hf[b], g1_sb[l], be1_sb[l])
                    mlp(b, l)
                    layernorm(Hf[b], g2_sb[l], be2_sb[l])
            for b in range(BL):
                readout(b)

    nc.compile()
    return nc


def _prep_inputs(inputs):
    f = lambda x: np.asarray(x, np.float32)
    bf = lambda x: np.ascontiguousarray(np.asarray(x, np.float32).astype(NPF16))
    xs = f(inputs["xs"])
    xsT = np.ascontiguousarray(xs.transpose(0, 2, 1)).astype(NPF16)  # [B, D, N]
    WoutT = np.ascontiguousarray(f(inputs["Wout"]).reshape(N, E).T)  # [E, N]

    def cols(v, per_l):
        v = f(v)
        if per_l:
            return np.ascontiguousarray(v.reshape(L, EC, P).transpose(0, 2, 1))
        return np.ascontiguousarray(v.reshape(EC, P).T)

    common = {
        "Win": bf(inputs["Win"]),
        "Wq": bf(inputs["Wq"]),
        "Wk": bf(inputs["Wk"]),
        "Wv": bf(inputs["Wv"]),
        "W1": bf(inputs["W1"]),
        "W2": bf(inputs["W2"]),
        "WoutT": WoutT,
        "colpack": np.concatenate(
            [cols(inputs["b_in"], False)]
            + [
                cols(inputs[k], True).transpose(1, 0, 2).reshape(P, L * EC)
                for k in ("bm1", "bm2", "be1", "be2", "g1", "g2")
            ],
            axis=1,
        ),
        "b_out": f(inputs["b_out"]).reshape(1, 1),
    }
    in_maps = []
    for c in range(NCORES):
        m = dict(common)
        m["xsT"] = np.ascontiguousarray(xsT[c * BL : (c + 1) * BL])
        in_maps.append(m)
    return in_maps


def get_program():
    if "nc" not in _CACHE:
        _CACHE["nc"] = _build()
    return _CACHE["nc"]


def kernel(**inputs) -> np.ndarray:
    nc = get_program()
    in_maps = _prep_inputs(inputs)
    res = run_bass_kernel_spmd(nc, in_maps, list(range(NCORES)))
    out = np.concatenate([res.results[c]["out"] for c in range(NCORES)], axis=0)
    return out.astype(np.float32)



# revision 3
# speedup vs baseline: 1.1637x; 1.1637x over previous
"""Trainium2 Bass kernel for nn_EncoderTransformer_61194694033513.

Data-parallel over batch B=16 across 8 NeuronCores (2 batch elems per core).
Per core the forward runs out of SBUF, activations feature-major HT[e, tok]
in fp16, interleaved-chunk column layout: col = c*1024 + ec*512 + t'
(c = tok//512, t' = tok%512, ec = feat//128, partition = feat%128), so every
per-chunk op is one contiguous [P,1024] instruction covering both feature
chunks.

Key structural points vs the v1 kernel:
- Wk is folded into Wqk = Wq @ Wk^T on the host; scores S = (H Wqk) H^T use
  the live H tiles as keys, eliminating the k projection matmuls and copies.
- H is double-buffered across layers (attention reads Hcur, writes Hcur+O
  into Hnxt), so no pristine-copy hazard.
- LN stats are REPLICATED across partitions via ones[P,128] lhsT matmuls
  ([P,512] PSUM rows instead of [1,512]), so all row math runs 128-lane wide
  and no gpsimd partition_broadcast is needed; rstd/mean*rstd feed the apply
  directly.
- The two batch elems are software-pipelined half a layer out of phase
  (A's attention overlaps B's LN/MLP chains and vice versa), emitted as
  proportionally interleaved task lists, so the PE queue always has ready
  matmul work and HAM never throttles.
- LN applies run on gpsimd (SBUF-only fp16), relu of scores alternates
  DVE/ACT, keeping all three element engines ~50% loaded under the PE.
"""

import sys

import numpy as np

for _p in (
    "/opt/trn_rl_repo",
    "/opt/pypackages",
    "/root/.axon_site",
    "/root/.axon_site/_ro/trn_rl_repo",
    "/root/.axon_site/_ro/pypackages",
):
    if _p not in sys.path:
        sys.path.append(_p)

import concourse.bass as bass  # noqa: E402
import concourse.bacc as bacc  # noqa: E402
import concourse.mybir as mybir  # noqa: E402
from concourse import tile  # noqa: E402
from concourse.bass_utils import run_bass_kernel_spmd  # noqa: E402

B, N, D, E, L = 16, 2048, 128, 256, 3
NCORES = 8
BL = B // NCORES
P = 128
EC = E // P  # 2 feature chunks
CH = N // 512  # 4 token chunks
JT = N // P  # 16 key tiles
EPS = 1e-5
F32 = mybir.dt.float32
F16 = mybir.dt.float16
NPF16 = np.float16
AF = mybir.ActivationFunctionType
OP = mybir.AluOpType

_CACHE = {}


def _col(c, ec, off=0):
    """H-space column index for token chunk c, feature chunk ec."""
    return c * 1024 + ec * 512 + off


def _merge(xs, ys):
    """Proportionally interleave two task lists (Bresenham)."""
    n, m = len(xs), len(ys)
    out = []
    i = j = 0
    while i < n or j < m:
        if j >= m or (i < n and i * (m + 1) <= j * (n + 1)):
            out.append(xs[i])
            i += 1
        else:
            out.append(ys[j])
            j += 1
    return out


def _build():
    nc = bacc.Bacc("TRN2", target_bir_lowering=False, debug=False, num_devices=NCORES)

    d_xsT = nc.declare_dram_parameter("xsT", [BL, P, N], F16, isOutput=False)
    d_Win = nc.declare_dram_parameter("Win", [D, E], F16, isOutput=False)
    WNAMES = ("Wqk", "Wv", "W1", "W2")
    d_W = {
        nm: nc.declare_dram_parameter(nm, [L, E, E], F16, isOutput=False)
        for nm in WNAMES
    }
    d_woutP = nc.declare_dram_parameter("woutP", [P, 2 * N], F16, isOutput=False)
    NCOL = 2 + 8 * L * EC
    d_colpack = nc.declare_dram_parameter("colpack", [P, NCOL], F32, isOutput=False)
    d_bout = nc.declare_dram_parameter("b_out", [1, 1], F32, isOutput=False)
    d_out = nc.declare_dram_parameter("out", [BL, 1], F32, isOutput=True)

    with tile.TileContext(nc) as tc:
        from contextlib import ExitStack

        with ExitStack() as ctx:
            cpool = ctx.enter_context(tc.tile_pool(name="const", bufs=1))
            hpool = ctx.enter_context(tc.tile_pool(name="acts", bufs=1))
            sqpool = ctx.enter_context(tc.tile_pool(name="sqp", bufs=2))
            spool = ctx.enter_context(tc.tile_pool(name="srelu", bufs=3))
            apool = ctx.enter_context(tc.tile_pool(name="mlpa", bufs=2))
            rpool = ctx.enter_context(tc.tile_pool(name="rowm", bufs=2))
            uppool = ctx.enter_context(tc.tile_pool(name="applyp", bufs=4))
            ropool = ctx.enter_context(tc.tile_pool(name="ro", bufs=2))

            PS = bass.MemorySpace.PSUM
            ps_s = ctx.enter_context(tc.tile_pool(name="ps_s", bufs=2, space=PS))
            ps_o = ctx.enter_context(tc.tile_pool(name="ps_o", bufs=1, space=PS))
            ps_d = ctx.enter_context(tc.tile_pool(name="ps_d", bufs=1, space=PS))

            # ---- input DMAs first (chunked so proj can start early) ------
            xs_tiles = []
            for b in range(BL):
                xt = hpool.tile([P, N], F16, name=f"xst{b}", tag=f"xst{b}")
                for c in range(CH):
                    cs = slice(c * 512, (c + 1) * 512)
                    nc.sync.dma_start(xt[:, cs], d_xsT[b][:, cs])
                xs_tiles.append(xt)

            win_sb = cpool.tile([P, E], F16, name="win", tag="win")
            nc.sync.dma_start(win_sb[:], d_Win[:])
            colpack = cpool.tile([P, NCOL], F32, name="colpack", tag="colpack")
            nc.sync.dma_start(colpack[:], d_colpack[:])
            binp_sb = colpack[:, 0:EC]

            def col_views(base):
                return [
                    colpack[
                        :, 2 + base * L * EC + l * EC : 2 + base * L * EC + (l + 1) * EC
                    ]
                    for l in range(L)
                ]

            bm1_sb = col_views(0)
            bm2_sb = col_views(1)
            be1_sb = col_views(2)
            be2_sb = col_views(3)
            g1_sb = col_views(4)
            g2_sb = col_views(5)
            ng1_sb = col_views(6)
            ng2_sb = col_views(7)

            # weights: layer-major DMA issue order so layer 0 lands first
            w_sb = {nm: [None] * L for nm in WNAMES}
            w_big = {}
            for nm in WNAMES:
                w_big[nm] = [
                    cpool.tile([P, L * E], F16, name=f"{nm}B{ec}", tag=f"{nm}B{ec}")
                    for ec in range(EC)
                ]
            for l in range(L):
                for nm in WNAMES:
                    for ec in range(EC):
                        nc.sync.dma_start(
                            w_big[nm][ec][:, l * E : (l + 1) * E],
                            d_W[nm][l, ec * P : (ec + 1) * P, :],
                        )
            for nm in WNAMES:
                for l in range(L):
                    w_sb[nm][l] = [
                        w_big[nm][ec][:, l * E : (l + 1) * E] for ec in range(EC)
                    ]

            wout_sb = cpool.tile([P, 2 * N], F16, name="woutp", tag="woutp")
            nc.sync.dma_start(wout_sb[:], d_woutP[:])
            bout_sb = cpool.tile([1, 1], F32, name="bout", tag="bout")
            nc.sync.dma_start(bout_sb[:], d_bout[:])

            ones128 = cpool.tile([P, P], F16, name="ones128", tag="ones128")
            nc.vector.memset(ones128[:], 1.0)
            epsc = cpool.tile([P, 1], F32, name="epsc", tag="epsc")
            nc.vector.memset(epsc[:], EPS)

            # ---- persistent per-elem tiles -------------------------------
            Hbuf = [
                [
                    hpool.tile([P, 2 * N], F16, name=f"H{b}{i}", tag=f"H{b}{i}")
                    for i in range(2)
                ]
                for b in range(BL)
            ]
            qT = [hpool.tile([P, 2 * N], F16, name=f"qT{b}", tag=f"qT{b}") for b in range(BL)]
            v_sb = [
                hpool.tile([P, JT * E], F16, name=f"v{b}", tag=f"v{b}")
                for b in range(BL)
            ]
            rstd_all = [
                hpool.tile([P, N], F16, name=f"rstd{b}", tag=f"rstd{b}")
                for b in range(BL)
            ]
            mrstd_all = [
                hpool.tile([P, N], F16, name=f"mrstd{b}", tag=f"mrstd{b}")
                for b in range(BL)
            ]

            relu_ctr = [0, 0]

            # ---- stage task builders (each task = one emission closure) --
            def proj_tasks(b):
                def mk(c):
                    def t():
                        ps = ps_d.tile([P, 1024], F32, name="psp", tag="d")
                        for ec in range(EC):
                            nc.tensor.matmul(
                                ps[:, ec * 512 : (ec + 1) * 512],
                                win_sb[:, ec * P : (ec + 1) * P],
                                xs_tiles[b][:, c * 512 : (c + 1) * 512],
                            )
                        for ec in range(EC):
                            nc.vector.tensor_scalar_add(
                                Hbuf[b][0][:, _col(c, ec) : _col(c, ec) + 512],
                                ps[:, ec * 512 : (ec + 1) * 512],
                                binp_sb[:, ec : ec + 1],
                            )

                    return t

                return [mk(c) for c in range(CH)]

            def qv_tasks(b, l):
                cur = Hbuf[b][l % 2]
                tasks = []

                def mk_q(c):
                    def t():
                        ps = ps_d.tile([P, 1024], F32, name="psq", tag="d")
                        for dc in range(EC):
                            for ec in range(EC):
                                nc.tensor.matmul(
                                    ps[:, dc * 512 : (dc + 1) * 512],
                                    w_sb["Wqk"][l][ec][:, dc * P : (dc + 1) * P],
                                    cur[:, _col(c, ec) : _col(c, ec) + 512],
                                    start=(ec == 0),
                                    stop=(ec == EC - 1),
                                )
                        nc.scalar.copy(
                            qT[b][:, c * 1024 : (c + 1) * 1024], ps[:]
                        )

                    return t

                def mk_v(t2):
                    def t():
                        ps = ps_d.tile([P, 1024], F32, name="psv", tag="d")
                        for h in range(2):
                            kt = 2 * t2 + h
                            for ec in range(EC):
                                nc.tensor.matmul(
                                    ps[:, h * E : (h + 1) * E],
                                    cur[
                                        :,
                                        _col(kt // 4, ec, (kt % 4) * P) : _col(
                                            kt // 4, ec, (kt % 4) * P
                                        )
                                        + P,
                                    ],
                                    w_sb["Wv"][l][ec][:],
                                    start=(ec == 0),
                                    stop=(ec == EC - 1),
                                )
                        dst = v_sb[b][:, 2 * t2 * E : (2 * t2 + 2) * E]
                        if t2 % 2 == 0:
                            nc.scalar.copy(dst, ps[:, 0 : 2 * E])
                        else:
                            nc.vector.tensor_copy(dst, ps[:, 0 : 2 * E])

                    return t

                for c in range(CH):
                    tasks.append(mk_q(c))
                for t2 in range(JT // 2):
                    tasks.append(mk_v(t2))
                return tasks

            def attn_tasks(b, l):
                cur = Hbuf[b][l % 2]
                nxt = Hbuf[b][(l + 1) % 2]
                tasks = []
                state = {}

                def mk_j(c, j2):
                    def t():
                        if j2 == 0:
                            state["o"] = ps_o.tile([P, 1024], F32, name="o", tag="o")
                        o_ps = state["o"]
                        s_ps = ps_s.tile([P, 1024], F32, name="s", tag="s")
                        for h in range(2):
                            j = 2 * j2 + h
                            for dc in range(EC):
                                nc.tensor.matmul(
                                    s_ps[:, h * 512 : (h + 1) * 512],
                                    cur[
                                        :,
                                        _col(j // 4, dc, (j % 4) * P) : _col(
                                            j // 4, dc, (j % 4) * P
                                        )
                                        + P,
                                    ],
                                    qT[b][:, _col(c, dc) : _col(c, dc) + 512],
                                    start=(dc == 0),
                                    stop=(dc == EC - 1),
                                )
                        sr = spool.tile([P, 1024], F16, name="sr", tag="sr")
                        if relu_ctr[b] % 2 == 0:
                            nc.vector.tensor_relu(sr[:], s_ps[:])
                        else:
                            nc.scalar.activation(sr[:], s_ps[:], AF.Relu)
                        relu_ctr[b] += 1
                        for h in range(2):
                            j = 2 * j2 + h
                            for oc in range(EC):
                                nc.tensor.matmul(
                                    o_ps[:, oc * 512 : (oc + 1) * 512],
                                    v_sb[b][:, j * E + oc * P : j * E + (oc + 1) * P],
                                    sr[:, h * 512 : (h + 1) * 512],
                                    start=(j == 0),
                                    stop=(j == JT - 1),
                                )

                    return t

                def mk_add(c):
                    def t():
                        nc.vector.tensor_add(
                            nxt[:, c * 1024 : (c + 1) * 1024],
                            cur[:, c * 1024 : (c + 1) * 1024],
                            state["o"][:],
                        )

                    return t

                for c in range(CH):
                    for j2 in range(JT // 2):
                        tasks.append(mk_j(c, j2))
                    tasks.append(mk_add(c))
                return tasks

            def ln_tasks(b, X, g_col, ng_col, be_col):
                """X: [P,2N] tile, in-place LN over features."""
                tasks = []

                def mk_stats(c):
                    def t():
                        sq = sqpool.tile([P, 1024], F16, name="sq", tag="sq")
                        nc.scalar.activation(
                            sq[:],
                            X[:, c * 1024 : (c + 1) * 1024],
                            AF.Square,
                            scale=1.0 / 64,
                        )
                        st = ps_d.tile([P, 1024], F32, name="st", tag="d")
                        st_s = st[:, 0:512]
                        st_q = st[:, 512:1024]
                        for ec in range(EC):
                            nc.tensor.matmul(
                                st_s,
                                ones128[:],
                                X[:, _col(c, ec) : _col(c, ec) + 512],
                                start=(ec == 0),
                                stop=(ec == EC - 1),
                            )
                        for ec in range(EC):
                            nc.tensor.matmul(
                                st_q,
                                ones128[:],
                                sq[:, ec * 512 : (ec + 1) * 512],
                                start=(ec == 0),
                                stop=(ec == EC - 1),
                            )
                        cs = slice(c * 512, (c + 1) * 512)
                        t1 = rpool.tile([P, 512], F32, name="t1", tag="t1")
                        # t1 = (sum/64)^2 = sum^2/4096
                        nc.scalar.activation(t1[:], st_s, AF.Square, scale=1.0 / 64)
                        # t1 <- E*var/4096 = sumsq/4096 - sum^2/(E*4096)
                        nc.vector.scalar_tensor_tensor(
                            t1[:], t1[:], -1.0 / E, st_q, op0=OP.mult, op1=OP.add
                        )
                        nc.scalar.activation(
                            rstd_all[b][:, cs],
                            t1[:],
                            AF.Abs_reciprocal_sqrt,
                            bias=epsc[:],
                            scale=4096.0 / E,
                        )
                        nc.vector.scalar_tensor_tensor(
                            mrstd_all[b][:, cs],
                            st_s,
                            1.0 / E,
                            rstd_all[b][:, cs],
                            op0=OP.mult,
                            op1=OP.mult,
                        )

                    return t

                def mk_apply(c):
                    def t():
                        cs = slice(c * 512, (c + 1) * 512)
                        for pt in range(EC):
                            xs_ = slice(_col(c, pt), _col(c, pt) + 512)
                            up = uppool.tile([P, 512], F16, name="up", tag="up")
                            # up = -g*mean*rstd + be
                            nc.gpsimd.tensor_scalar(
                                up[:],
                                mrstd_all[b][:, cs],
                                ng_col[:, pt : pt + 1],
                                be_col[:, pt : pt + 1],
                                op0=OP.mult,
                                op1=OP.add,
                            )
                            tt = uppool.tile([P, 512], F16, name="tt", tag="tt")
                            # tt = (X*g)*rstd
                            nc.vector.scalar_tensor_tensor(
                                tt[:],
                                X[:, xs_],
                                g_col[:, pt : pt + 1],
                                rstd_all[b][:, cs],
                                op0=OP.mult,
                                op1=OP.mult,
                            )
                            nc.gpsimd.tensor_add(X[:, xs_], tt[:], up[:])

                    return t

                for c in range(CH):
                    tasks.append(mk_stats(c))
                for c in range(CH):
                    tasks.append(mk_apply(c))
                return tasks

            def mlp_tasks(b, l):
                X = Hbuf[b][(l + 1) % 2]

                def mk(c):
                    def t():
                        psa = ps_d.tile([P, 1024], F32, name="psa", tag="d")
                        for mc in range(EC):
                            for ec in range(EC):
                                nc.tensor.matmul(
                                    psa[:, mc * 512 : (mc + 1) * 512],
                                    w_sb["W1"][l][ec][:, mc * P : (mc + 1) * P],
                                    X[:, _col(c, ec) : _col(c, ec) + 512],
                                    start=(ec == 0),
                                    stop=(ec == EC - 1),
                                )
                        a = apool.tile([P, 1024], F16, name="a", tag="a")
                        for mc in range(EC):
                            nc.scalar.activation(
                                a[:, mc * 512 : (mc + 1) * 512],
                                psa[:, mc * 512 : (mc + 1) * 512],
                                AF.Relu,
                                bias=bm1_sb[l][:, mc : mc + 1],
                            )
                        psm = ps_d.tile([P, 1024], F32, name="psm", tag="d")
                        for oc in range(EC):
                            for mc in range(EC):
                                nc.tensor.matmul(
                                    psm[:, oc * 512 : (oc + 1) * 512],
                                    w_sb["W2"][l][mc][:, oc * P : (oc + 1) * P],
                                    a[:, mc * 512 : (mc + 1) * 512],
                                    start=(mc == 0),
                                    stop=(mc == EC - 1),
                                )
                        for oc in range(EC):
                            xs_ = slice(_col(c, oc), _col(c, oc) + 512)
                            nc.vector.scalar_tensor_tensor(
                                X[:, xs_],
                                psm[:, oc * 512 : (oc + 1) * 512],
                                bm2_sb[l][:, oc : oc + 1],
                                X[:, xs_],
                                op0=OP.add,
                                op1=OP.add,
                            )

                    return t

                return [mk(c) for c in range(CH)]

            def dense_tasks(b, l):
                return (
                    ln_tasks(b, Hbuf[b][(l + 1) % 2], g1_sb[l], ng1_sb[l], be1_sb[l])
                    + mlp_tasks(b, l)
                    + ln_tasks(b, Hbuf[b][(l + 1) % 2], g2_sb[l], ng2_sb[l], be2_sb[l])
                )

            def ro_tasks(b):
                X = Hbuf[b][L % 2]

                def t1():
                    ros = ropool.tile([P, 2 * N], F16, name="ros", tag="ros")
                    nc.vector.tensor_mul(ros[:], X[:], wout_sb[:])
                    st = ps_d.tile([P, 1024], F32, name="str", tag="d")
                    for k in range(2 * CH):
                        nc.tensor.matmul(
                            st[:, 0:512],
                            ones128[:],
                            ros[:, k * 512 : (k + 1) * 512],
                            start=(k == 0),
                            stop=(k == 2 * CH - 1),
                        )
                    rsum = ropool.tile([P, 1], F32, name="rsum", tag="rsum")
                    nc.vector.reduce_sum(rsum[:], st[:, 0:512], axis=mybir.AxisListType.X)
                    ob = ropool.tile([1, 1], F32, name="ob", tag="ob")
                    nc.scalar.activation(
                        ob[:], rsum[0:1, :], AF.Identity, bias=bout_sb[:]
                    )
                    nc.sync.dma_start(d_out[b : b + 1, :], ob[:])

                return [t1]

            # ---- schedule: two streams half a layer out of phase ---------
            A, Bb = 0, 1
            sched = []
            sched += _merge(proj_tasks(A), proj_tasks(Bb))
            sched += _merge(qv_tasks(A, 0) + attn_tasks(A, 0), qv_tasks(Bb, 0))
            sched += _merge(dense_tasks(A, 0), attn_tasks(Bb, 0))
            sched += _merge(
                qv_tasks(A, 1) + attn_tasks(A, 1), dense_tasks(Bb, 0) + qv_tasks(Bb, 1)
            )
            sched += _merge(dense_tasks(A, 1), attn_tasks(Bb, 1))
            sched += _merge(
                qv_tasks(A, 2) + attn_tasks(A, 2), dense_tasks(Bb, 1) + qv_tasks(Bb, 2)
            )
            sched += _merge(dense_tasks(A, 2), attn_tasks(Bb, 2))
            sched += _merge(ro_tasks(A), dense_tasks(Bb, 2))
            sched += ro_tasks(Bb)
            for t in sched:
                t()

    nc.compile()
    return nc


def _prep_inputs(inputs):
    f = lambda x: np.asarray(x, np.float32)
    xs = f(inputs["xs"])
    xsT = np.ascontiguousarray(xs.transpose(0, 2, 1)).astype(NPF16)  # [B, D, N]
    Wq, Wk = f(inputs["Wq"]), f(inputs["Wk"])
    Wqk = np.stack([Wq[l] @ Wk[l].T for l in range(L)]).astype(NPF16)
    WoutT = f(inputs["Wout"]).reshape(N, E).T  # [E, N]
    woutP = np.ascontiguousarray(
        WoutT.reshape(EC, P, CH, 512).transpose(1, 2, 0, 3).reshape(P, 2 * N)
    ).astype(NPF16)

    def cols(v, per_l):
        v = f(v)
        if per_l:
            return np.ascontiguousarray(v.reshape(L, EC, P).transpose(0, 2, 1))
        return np.ascontiguousarray(v.reshape(EC, P).T)

    groups = [
        cols(inputs["bm1"], True),
        cols(inputs["bm2"], True),
        cols(inputs["be1"], True),
        cols(inputs["be2"], True),
        cols(inputs["g1"], True),
        cols(inputs["g2"], True),
        cols(-f(inputs["g1"]), True),
        cols(-f(inputs["g2"]), True),
    ]
    colpack = np.concatenate(
        [cols(inputs["b_in"], False)]
        + [g.transpose(1, 0, 2).reshape(P, L * EC) for g in groups],
        axis=1,
    )
    common = {
        "Win": f(inputs["Win"]).astype(NPF16),
        "Wqk": Wqk,
        "Wv": f(inputs["Wv"]).astype(NPF16),
        "W1": f(inputs["W1"]).astype(NPF16),
        "W2": f(inputs["W2"]).astype(NPF16),
        "woutP": woutP,
        "colpack": np.ascontiguousarray(colpack),
        "b_out": f(inputs["b_out"]).reshape(1, 1),
    }
    in_maps = []
    for c in range(NCORES):
        m = dict(common)
        m["xsT"] = np.ascontiguousarray(xsT[c * BL : (c + 1) * BL])
        in_maps.append(m)
    return in_maps


def get_program():
    if "nc" not in _CACHE:
        _CACHE["nc"] = _build()
    return _CACHE["nc"]


def kernel(**inputs) -> np.ndarray:
    nc = get_program()
    in_maps = _prep_inputs(inputs)
    res = run_bass_kernel_spmd(nc, in_maps, list(range(NCORES)))
    out = np.concatenate([res.results[c]["out"] for c in range(NCORES)], axis=0)
    return out.astype(np.float32)
